# revision 76
# baseline (speedup 1.0000x reference)
"""AlphaNet-v1 Trainium2 kernel: windowed stats + global BatchNorm + tiny MLP.

Strategy (data-parallel over batch, 8 cores):
  Phase 1 (device): per-sample raw features in f32 (corr/cov/std/z/dec/mean/
    ret over 3 windows, plus window max/min) -> raw[B,832] f32 in DRAM.
    x is DMA'd straight into the u-buffer and biased in place on ACT. All BN
    statistics (per-group sum / sum-of-squares) ride as accum_out
    side-channels of the ops that already produce the features plus a few ACT
    Square+accum passes -- no PE column sums. Pair products use a d-outer
    layout split into two half-pipelines (shifts 1-3 / 4-10) across DVE and
    GPSIMD; single-writer d-tree first levels for sum-sq and decay ride the
    idle DMA engines as SBUF->SBUF accumulate-DMAs (multi-writer product
    tiles reduce on DVE -- gpsimd accum-DMA over Pool-written column slices
    loses the write->read dependency).
  Host: combine partial sums -> BN affines for every reference column; fold
    the affine into W1'/b1' (wmean block folded into the direct columns).
  Phase 2 (device): per 128-sample chunk, PE-transpose raw to feature-major
    and matmul with SAMPLES as the stationary dim, h (32) as the moving dim:
    out free = 32 so each of the 28 accumulating f32 matmuls costs ~53 ns;
    the b1 bias is folded in via an all-ones feature row (col 832). relu on
    ACT, u-weighting and the 32-way h-reduction on DVE; y reassembled
    host-side (y = dev + c0). Everything stays exact f32 -- no fp32r in the
    value path. PSUM->SBUF copies alternate DVE/ACT under the DMA shadow.
"""

import numpy as np

import concourse.bass as bass
import concourse.bass_isa as bass_isa
import concourse.bacc as bacc
import concourse.mybir as mybir
from concourse.tile import TileContext
from concourse.bass_utils import run_bass_kernel_spmd

F32 = mybir.dt.float32
F32R = mybir.dt.float32r
ALU = mybir.AluOpType
AF = mybir.ActivationFunctionType

B = 131072
NCORES = 8
BS = B // NCORES            # 16384 samples per core
S = 8                       # samples per partition row per tile
PT = 128
TSAMP = PT * S              # 1024 samples per tile
NT = BS // TSAMP            # 16 tiles
NF, NW, ND = 11, 3, 10
XC = NF * NW * ND           # 330
RC = 832                    # raw feature columns stored (825 used + 7 pad)
NSL = 56                    # PSALL stat slots
EPS = 1e-5

# feature blocks in the 165-feature packing (f-major, w-minor in raw cols):
# corr 0:55, cov 55:110, std 110:121, z 121:132, dec 132:143, mean 143:154,
# ret 154:165
GF = [(0, 55), (55, 110), (110, 121), (121, 132), (132, 143), (143, 154), (154, 165)]
GCNT = np.array([55, 55, 11, 11, 11, 11, 11], dtype=np.int64)
# (alpha, beta): true_feature = alpha*raw + beta  (this kernel's raw units)
GAB = [(1.0, 0.0), (1.0 / 9.0, 0.0), (1.0 / 3.0, 0.0), (0.3, 0.0),
       (1.0 / 55.0, 1.0), (0.1, 1.0), (1.0, 0.0)]
# pair-block offsets for shifted products d=1..10 within 55-pair packing
OFFD = np.cumsum([0] + [11 - d for d in range(1, 11)]).tolist()

# PSALL slot map
SL_S1 = {"mean": 0, "dec": 1, "corr": 2, "cov": 3, "std": 4, "z": 5, "ret": 6}
SL_S2 = {"mean": 7, "std": 8, "corr": 9, "cov": 10, "dec": 11, "z": 12, "ret": 13}
SL_S1_WMAX = 14   # +g
SL_S2_WMAX = 21
SL_S1_WMIN = 28
SL_S2_WMIN = 35
SL_S2_WM = 42
SL_S1_COVB = 49
SL_S1_CORRB = 50

# DVE/Pool split knob: DVE handles samples [0:SA), Pool [SA:S)
SA = 1
# product shifts computed on DVE (rest go to GPSIMD)
PROD_DVE = {1, 2, 3}


def _feat_perm():
    """map my 165-feature index -> reference feature index (triu pair order)."""
    iu, ju = np.triu_indices(NF, k=1)
    ref_of_pair = {(i, j): k for k, (i, j) in enumerate(zip(iu, ju))}
    fmap = np.arange(165)
    for d in range(1, 11):
        for f in range(NF - d):
            mine = OFFD[d - 1] + f
            ref = ref_of_pair[(f, f + d)]
            fmap[mine] = ref           # corr block
            fmap[55 + mine] = 55 + ref  # cov block
    return fmap


FMAP = _feat_perm()
GOF = np.concatenate([np.full(GCNT[g], g) for g in range(7)])  # feat -> group


def build_phase1():
    nc = bacc.Bacc()
    x_in = nc.dram_tensor("x_in", [BS, XC], F32, kind="ExternalInput")
    wb_in = nc.dram_tensor("wb_in", [PT, XC], F32, kind="ExternalInput")
    raw_out = nc.dram_tensor("raw_out", [BS, RC], F32, kind="ExternalOutput")
    ps_out = nc.dram_tensor("ps_out", [PT, NT, NSL], F32, kind="ExternalOutput")

    x_t = x_in.rearrange("(t p s) c -> t p (s c)", t=NT, p=PT, s=S)
    raw_t = raw_out.rearrange("(t p s) c -> t p (s c)", t=NT, p=PT, s=S)

    # raw F3 block columns (f-major, w-minor): feature f spans cols 3f..3f+2
    C_CORR, C_COV, C_STD, C_Z, C_DEC, C_MEAN, C_RET = (0, 165, 330, 363, 396,
                                                       429, 462)

    with TileContext(nc) as tc, \
         nc.allow_low_precision(reason="f32 pipeline; stats via accum"):
        with tc.tile_pool(name="cst", bufs=1) as cp, \
             tc.tile_pool(name="io", bufs=2) as iop, \
             tc.tile_pool(name="wk", bufs=1) as wp, \
             tc.tile_pool(name="wk1", bufs=1) as wp1, \
             tc.tile_pool(name="sq", bufs=1) as sp, \
             tc.tile_pool(name="big", bufs=1) as bp:

            WB = cp.tile([PT, XC], F32)
            nc.sync.dma_start(WB[:], wb_in[:, :])
            WBdo1 = WB.rearrange("p (fw d) -> p d fw", d=ND)
            PSALL = cp.tile([PT, NT, NSL], F32)
            nc.vector.memset(PSALL[:], 0.0)

            def psl(k):
                return PS[:, k:k + 1]

            Ut = []
            prev = None
            for tp in range(2):
                Up = bp.tile([PT, S * XC], F32, tag="U", bufs=3)
                nc.sync.dma_start(Up[:], x_t[tp])
                Ut.append(Up)
            for t in range(NT):
                PS = PSALL[:, t, :]
                if prev is not None:
                    pRAW, pt = prev
                    hv = pRAW.rearrange("p (h x) -> p h x", h=2)
                    rv2 = raw_t[pt].rearrange("p (h x) -> p h x", h=2)
                    nc.sync.dma_start(rv2[:, 0], hv[:, 0])
                    nc.sync.dma_start(rv2[:, 1], hv[:, 1])
                if t + 2 < NT:
                    Un = bp.tile([PT, S * XC], F32, tag="U", bufs=3)
                    nc.sync.dma_start(Un[:], x_t[t + 2])
                    Ut.append(Un)
                U = Ut[t]

                RAW = iop.tile([PT, S * RC], F32, tag="RAW")
                RV = RAW.rearrange("p (s c) -> p s c", s=S)
                F3 = RV[:, :, 0:495].rearrange("p s (f w) -> p s f w", w=NW)
                nc.gpsimd.memset(RV[:, :, 825:RC], 0.0)

                # u = x - 1 in place (accum -> sum u = S1_mean raw units)
                nc.scalar.activation(U[:], U[:], AF.Copy, bias=-1.0,
                                     accum_out=psl(SL_S1["mean"]))
                Udo = U.rearrange("p (g d) -> p d g", d=ND)        # d-outer view
                Ud3 = U.rearrange("p (s fw d) -> p d s fw", fw=NF * NW, d=ND)

                # shifted pair products, two independent half-pipelines:
                # PA (delta 1-3, kw 0:81), PB (delta 4-10, kw 81:165); d-tree
                # L1/L2 ride the idle DMA engines (gpsimd accum DMA)
                PA = bp.tile([PT, ND, S, 81], F32, tag="PA", bufs=1)
                PB = bp.tile([PT, ND, S, 84], F32, tag="PB", bufs=1)
                for d in range(1, 11):
                    n3 = (NF - d) * 3
                    if d <= 3:
                        tgt = PA[:, :, :, OFFD[d - 1] * 3:OFFD[d - 1] * 3 + n3]
                    else:
                        o3 = (OFFD[d - 1] - 27) * 3
                        tgt = PB[:, :, :, o3:o3 + n3]
                    if d in PROD_DVE:
                        nc.vector.tensor_tensor(tgt, Ud3[:, :, :, 0:n3],
                                                Ud3[:, :, :, 3 * d:3 * d + n3],
                                                ALU.mult)
                    else:
                        nc.gpsimd.tensor_tensor(tgt, Ud3[:, :, :, 0:n3],
                                                Ud3[:, :, :, 3 * d:3 * d + n3],
                                                ALU.mult)
                SSh = []
                for hi, Ph in enumerate((PA, PB)):
                    Pw = Ph.rearrange("p d s kw -> p d (s kw)")
                    nc.vector.tensor_tensor(Pw[:, 0:5], Pw[:, 0:5],
                                            Pw[:, 5:10], ALU.add)
                    nc.vector.tensor_tensor(Pw[:, 0:2], Pw[:, 0:2], Pw[:, 2:4],
                                            ALU.add)
                    nc.vector.tensor_tensor(Pw[:, 0:1], Pw[:, 0:1], Pw[:, 1:2],
                                            ALU.add)
                    nc.vector.tensor_tensor(Pw[:, 0:1], Pw[:, 0:1], Pw[:, 4:5],
                                            ALU.add)
                    SSh.append(Ph[:, 0])

                # S2 = sum u^2 per window: ACT square (d-outer), DMA L1
                SQ2 = bp.tile([PT, ND, S * 33], F32, tag="SQ", bufs=1)
                nc.scalar.activation(SQ2[:], Udo, AF.Square)
                nc.gpsimd.dma_start(SQ2[:, 0:5], SQ2[:, 5:10], accum_op=ALU.add)
                nc.gpsimd.tensor_tensor(SQ2[:, 0:2], SQ2[:, 0:2], SQ2[:, 2:4],
                                        ALU.add)
                nc.gpsimd.tensor_tensor(SQ2[:, 0:1], SQ2[:, 0:1], SQ2[:, 1:2],
                                        ALU.add)
                S2C = wp.tile([PT, S, NF * NW], F32, tag="S2C")
                nc.vector.scalar_tensor_tensor(
                    S2C.rearrange("p s fw -> p (s fw)"), SQ2[:, 0], 1.0,
                    SQ2[:, 4], ALU.mult, ALU.add)

                # mean feature raw = MS = sum_d u via tree adds (L1 on Pool)
                MH2 = wp.tile([PT, 5, S * 33], F32, tag="MH", bufs=1)
                nc.gpsimd.tensor_tensor(MH2[:], Udo[:, 0:5], Udo[:, 5:10],
                                        ALU.add)
                nc.vector.tensor_tensor(MH2[:, 0:2], MH2[:, 0:2], MH2[:, 2:4],
                                        ALU.add)
                nc.vector.tensor_tensor(MH2[:, 0:1], MH2[:, 0:1], MH2[:, 1:2],
                                        ALU.add)
                M33 = RV[:, :, C_MEAN:C_MEAN + 33]
                nc.vector.scalar_tensor_tensor(
                    M33, MH2[:, 0].rearrange("p (s fw) -> p s fw", s=S), 1.0,
                    MH2[:, 4].rearrange("p (s fw) -> p s fw", s=S),
                    ALU.mult, ALU.add)

                # VT = 10 m^2 (accum -> 10*S2_mean); VARP = S2 - VT = M2
                VT = wp.tile([PT, S, 33], F32, tag="VT")
                nc.vector.scalar_tensor_tensor(VT[:], M33, 0.1, M33,
                                               ALU.mult, ALU.mult,
                                               accum_out=psl(SL_S2["mean"]))
                VARP = wp.tile([PT, S, 33], F32, tag="VARP")
                nc.vector.scalar_tensor_tensor(VARP[:], S2C[:], 1.0, VT[:],
                                               ALU.mult, ALU.subtract,
                                               accum_out=psl(SL_S2["std"]))

                # std = sqrt(M2) into F3 (accum -> S1_std); rstd = 1/std
                STD33 = RV[:, :, C_STD:C_STD + 33]
                nc.scalar.activation(STD33, VARP[:], AF.Sqrt,
                                     accum_out=psl(SL_S1["std"]))
                RSTD = wp.tile([PT, S, 33], F32, tag="RSTD")
                nc.vector.reciprocal(RSTD[:], STD33)

                # MM = 10 m_i m_j ; cov' = SS - MM (accum -> S1_cov)
                MM = wp1.tile([PT, S, 165], F32, tag="MM")
                for d in range(1, 11):
                    o, n = OFFD[d - 1] * 3, (NF - d) * 3
                    nc.vector.scalar_tensor_tensor(MM[:, :, o:o + n],
                                                   M33[:, :, 0:n], 0.1,
                                                   M33[:, :, 3 * d:3 * d + n],
                                                   ALU.mult, ALU.mult)
                nc.vector.scalar_tensor_tensor(RV[:, :, C_COV:C_COV + 81],
                                               SSh[0], 1.0, MM[:, :, 0:81],
                                               ALU.mult, ALU.subtract,
                                               accum_out=psl(SL_S1["cov"]))
                nc.vector.scalar_tensor_tensor(RV[:, :, C_COV + 81:C_COV + 165],
                                               SSh[1], 1.0, MM[:, :, 81:165],
                                               ALU.mult, ALU.subtract,
                                               accum_out=psl(SL_S1_COVB))

                # RR = rstd_i rstd_j ; corr = cov' * RR (accum -> S1_corr)
                RR = wp1.tile([PT, S, 165], F32, tag="RR")
                for d in range(1, 11):
                    o, n = OFFD[d - 1] * 3, (NF - d) * 3
                    nc.vector.scalar_tensor_tensor(RR[:, :, o:o + n],
                                                   RSTD[:, :, 0:n], 1.0,
                                                   RSTD[:, :, 3 * d:3 * d + n],
                                                   ALU.mult, ALU.mult)
                nc.vector.scalar_tensor_tensor(RV[:, :, C_CORR:C_CORR + 81],
                                               RV[:, :, C_COV:C_COV + 81], 1.0,
                                               RR[:, :, 0:81], ALU.mult, ALU.mult,
                                               accum_out=psl(SL_S1["corr"]))
                nc.vector.scalar_tensor_tensor(RV[:, :, C_CORR + 81:C_CORR + 165],
                                               RV[:, :, C_COV + 81:C_COV + 165],
                                               1.0, RR[:, :, 81:165], ALU.mult,
                                               ALU.mult,
                                               accum_out=psl(SL_S1_CORRB))

                # z = (m + 1) * rstd (accum -> S1_z)
                nc.vector.scalar_tensor_tensor(RV[:, :, C_Z:C_Z + 33], M33, 10.0,
                                               RSTD[:], ALU.add, ALU.mult,
                                               accum_out=psl(SL_S1["z"]))

                # ret = x9/x0 - 1 = (u9 - u0)/(1 + u0) (accum -> S1_ret)
                Ufw = U.rearrange("p (s f w d) -> p s f w d", s=S, f=NF, w=NW,
                                  d=ND)
                U0v = Ufw[:, :, :, :, 0].rearrange("p s f w -> p s (f w)")
                U9v = Ufw[:, :, :, :, 9].rearrange("p s f w -> p s (f w)")
                T0 = wp.tile([PT, S, 33], F32, tag="T0")
                nc.vector.tensor_scalar(T0[:], U0v, 1.0, None, ALU.add)
                R0 = wp.tile([PT, S, 33], F32, tag="R0")
                nc.vector.reciprocal(R0[:], T0[:])
                D90 = wp.tile([PT, S, 33], F32, tag="D90")
                nc.gpsimd.tensor_tensor(D90[:], U9v, U0v, ALU.subtract)
                nc.vector.scalar_tensor_tensor(RV[:, :, C_RET:C_RET + 33], R0[:],
                                               1.0, D90[:], ALU.mult, ALU.mult,
                                               accum_out=psl(SL_S1["ret"]))

                # dec = sum (d+1) u_d : weighted mult (Pool) then tree
                Dw2 = bp.tile([PT, ND, S * 33], F32, tag="DW", bufs=1)
                for si in range(S):
                    nc.gpsimd.tensor_tensor(Dw2[:, :, si * 33:(si + 1) * 33],
                                            Udo[:, :, si * 33:(si + 1) * 33],
                                            WBdo1, ALU.mult)
                nc.gpsimd.dma_start(Dw2[:, 0:5], Dw2[:, 5:10], accum_op=ALU.add)
                nc.gpsimd.tensor_tensor(Dw2[:, 0:2], Dw2[:, 0:2], Dw2[:, 2:4],
                                        ALU.add)
                nc.gpsimd.tensor_tensor(Dw2[:, 0:1], Dw2[:, 0:1], Dw2[:, 1:2],
                                        ALU.add)
                nc.vector.scalar_tensor_tensor(
                    RV[:, :, C_DEC:C_DEC + 33],
                    Dw2[:, 0].rearrange("p (s fw) -> p s fw", s=S), 1.0,
                    Dw2[:, 4].rearrange("p (s fw) -> p s fw", s=S),
                    ALU.mult, ALU.add, accum_out=psl(SL_S1["dec"]))

                # window max / min / sum over the 3 windows (strided w-views)
                FW0 = RV[:, :, 0:495].rearrange("p s (f w) -> p s f w", w=NW)
                F0 = FW0[:, :, :, 0]
                F1 = FW0[:, :, :, 1]
                F2 = FW0[:, :, :, 2]
                T01 = wp1.tile([PT, S, 165], F32, tag="T01")
                nc.vector.tensor_tensor(T01[:], F0[:], F1[:], ALU.max)
                T02 = wp1.tile([PT, S, 165], F32, tag="T02")
                nc.vector.tensor_tensor(T02[:], F0[:], F1[:], ALU.min)
                WMT = wp1.tile([PT, S, 165], F32, tag="WMT", bufs=1)
                nc.gpsimd.tensor_tensor(WMT[:], F0[:], F1[:], ALU.add)
                # finals per group with accum (wmax/wmin); wsum in place
                for g, (a, b) in enumerate(GF):
                    nc.vector.scalar_tensor_tensor(
                        RV[:, :, 495 + a:495 + b], T01[:, :, a:b], 1.0,
                        F2[:, :, a:b], ALU.mult, ALU.max,
                        accum_out=psl(SL_S1_WMAX + g))
                    nc.vector.scalar_tensor_tensor(
                        RV[:, :, 660 + a:660 + b], T02[:, :, a:b], 1.0,
                        F2[:, :, a:b], ALU.mult, ALU.min,
                        accum_out=psl(SL_S1_WMIN + g))
                nc.gpsimd.tensor_tensor(WMT[:], WMT[:], F2[:], ALU.add)

                # stat squares on ACT (accum -> S2 slots)
                def sq_acc(src, slot, tag, n):
                    scr = sp.tile([PT, n], F32, tag=tag)
                    sv = scr.rearrange("p (s x) -> p s x", s=S)
                    nc.scalar.activation(sv, src, AF.Square,
                                         accum_out=psl(slot))

                sq_acc(RV[:, :, C_CORR:C_CORR + 165], SL_S2["corr"], "SCRa",
                       S * 165)
                sq_acc(RV[:, :, C_COV:C_COV + 165], SL_S2["cov"], "SCRa",
                       S * 165)
                sq_acc(RV[:, :, C_DEC:C_DEC + 33], SL_S2["dec"], "SCRb", S * 33)
                sq_acc(RV[:, :, C_Z:C_Z + 33], SL_S2["z"], "SCRb", S * 33)
                sq_acc(RV[:, :, C_RET:C_RET + 33], SL_S2["ret"], "SCRb", S * 33)
                for g, (a, b) in enumerate(GF):
                    n = S * (b - a)
                    sq_acc(RV[:, :, 495 + a:495 + b], SL_S2_WMAX + g, "SCRc", n)
                    sq_acc(RV[:, :, 660 + a:660 + b], SL_S2_WMIN + g, "SCRc", n)
                    sq_acc(WMT[:, :, a:b], SL_S2_WM + g, "SCRc", n)

                prev = (RAW, t)
            pRAW, pt = prev
            nc.sync.dma_start(raw_t[pt], pRAW[:])
            nc.sync.dma_start(ps_out[:, :, :], PSALL[:])

    return nc


def build_phase2():
    nc = bacc.Bacc()
    raw_in = nc.dram_tensor("raw_in", [BS, RC], F32, kind="ExternalInput")
    w1t_in = nc.dram_tensor("w1t_in", [896, 32], F32, kind="ExternalInput")
    ub_in = nc.dram_tensor("ub_in", [PT, PT], F32, kind="ExternalInput")
    id_in = nc.dram_tensor("id_in", [PT, PT], F32, kind="ExternalInput")
    y_out = nc.dram_tensor("y_out", [PT, PT], F32, kind="ExternalOutput")

    NB = BS // 512  # 32 blocks of 512 samples
    rb = raw_in.rearrange("(n t p) c -> n p t c", n=NB, t=4, p=PT)

    with TileContext(nc) as tc:
        with tc.tile_pool(name="cst", bufs=1) as cp, \
             tc.tile_pool(name="sb", bufs=4) as sp, \
             tc.tile_pool(name="ft", bufs=3) as fp, \
             tc.tile_pool(name="ps", bufs=2, space="PSUM") as pp, \
             tc.tile_pool(name="ps2", bufs=2, space="PSUM") as pp2:
            W1S = cp.tile([PT, 7 * 32], F32)
            W1Sv = W1S.rearrange("p (c m) -> p c m", c=7)
            nc.sync.dma_start(W1Sv, w1t_in.rearrange("(c p) m -> p c m", c=7, p=PT))
            UB = cp.tile([PT, PT], F32)
            nc.sync.dma_start(UB[:], ub_in[:, :])
            IDT = cp.tile([PT, PT], F32)
            nc.sync.dma_start(IDT[:], id_in[:, :])

            YALL = cp.tile([PT, PT], F32)
            copy_rr = 0
            for n in range(NB):
                FTS = fp.tile([PT, 7, 512], F32, tag="FTS")
                Ft4 = sp.tile([PT, 4, RC], F32, tag="Ft")
                nc.sync.dma_start(Ft4[:], rb[n])
                for t in range(4):
                    Ft = Ft4[:, t]
                    bankA = pp.tile([PT, 512], F32, tag="A")
                    bankB = pp.tile([PT, 384], F32, tag="B")
                    for c in range(6):
                        tgt = (bankA[:, (c % 4) * 128:(c % 4) * 128 + 128] if c < 4
                               else bankB[:, (c - 4) * 128:(c - 4) * 128 + 128])
                        nc.tensor.transpose(tgt, Ft[:, c * PT:(c + 1) * PT], IDT[:])
                    nc.tensor.transpose(bankB[0:64, 256:384], Ft[:, 768:832], IDT[:])
                    outA = FTS[:, 0:4, t * 128:(t + 1) * 128]
                    outB = FTS[:, 4:7, t * 128:(t + 1) * 128]
                    for tgt, src, nchunk in ((outA, bankA, 4), (outB, bankB, 3)):
                        sv = src.rearrange("p (c x) -> p c x", c=nchunk)
                        eng = copy_rr % 2
                        copy_rr += 1
                        if eng == 0:
                            nc.vector.tensor_copy(tgt, sv)
                        else:
                            nc.scalar.copy(tgt, sv)
                # ones row for the folded bias (chunk 6 row 64 = col 832)
                nc.vector.memset(FTS[64:65, 6, :], 1.0)
                HS = sp.tile([PT, 4, 32], F32, tag="HS")
                HPT = pp2.tile([PT, 4, 32], F32, tag="HPT")
                for t in range(4):
                    for c in range(7):
                        kc = 65 if c == 6 else 128
                        nc.tensor.matmul(HPT[:, t], FTS[0:kc, c, t * 128:(t + 1) * 128],
                                         W1Sv[0:kc, c, :],
                                         start=(c == 0), stop=(c == 6))
                nc.scalar.activation(HS[:], HPT[:], AF.Relu)
                HSu = sp.tile([PT, 4, 32], F32, tag="HSu")
                nc.vector.tensor_tensor(HSu[:], HS[:],
                                        UB.rearrange("p (t j) -> p t j", t=4),
                                        ALU.mult)
                nc.vector.tensor_reduce(YALL[:, n * 4:(n + 1) * 4], HSu[:],
                                        mybir.AxisListType.X, ALU.add)
            nc.sync.dma_start(y_out[:, :], YALL[:])
    return nc


_CACHE = {}
LAST_EXEC_NS = []
LAST_TRACES = []


def _get_kernels():
    if "p1" not in _CACHE:
        _CACHE["p1"] = build_phase1()
        _CACHE["p1"].finalize()
        _CACHE["p2"] = build_phase2()
        _CACHE["p2"].finalize()
    return _CACHE["p1"], _CACHE["p2"]


def _decay_pattern():
    wb = np.zeros((NF, NW, ND), np.float32)
    wb[:, :, :] = np.arange(1, ND + 1, dtype=np.float32)
    full = np.broadcast_to(wb.reshape(1, -1), (PT, XC))
    return np.ascontiguousarray(full)


def kernel(x, gamma, beta, W1, b1, W2, b2, w_scale, b_scale):
    x = np.asarray(x, dtype=np.float32)
    W1 = np.asarray(W1, np.float32); b1 = np.asarray(b1, np.float32)
    W2 = np.asarray(W2, np.float32); b2 = np.asarray(b2, np.float32)
    gamma_f = float(np.asarray(gamma).reshape(-1)[0])
    beta_f = float(np.asarray(beta).reshape(-1)[0])
    wsc = float(np.asarray(w_scale).reshape(-1)[0])
    bsc = float(np.asarray(b_scale).reshape(-1)[0])

    nc1, nc2 = _get_kernels()
    xs = np.ascontiguousarray(x.reshape(B, XC))
    wb = _decay_pattern()

    LAST_EXEC_NS.clear()
    LAST_TRACES.clear()
    in1 = [{"x_in": xs[c * BS:(c + 1) * BS], "wb_in": wb}
           for c in range(NCORES)]
    r1 = run_bass_kernel_spmd(nc1, in1, core_ids=list(range(NCORES)))
    if r1.exec_time_ns is not None:
        LAST_EXEC_NS.append(("phase1", r1.exec_time_ns))
        if r1.instructions_and_trace:
            LAST_TRACES.append(r1.instructions_and_trace[1])
    raws = [r["raw_out"] for r in r1.results]

    # aggregate stat slots over cores / partitions / tiles
    P = np.zeros(NSL, np.float64)
    for r in r1.results:
        P += r["ps_out"].astype(np.float64).sum(axis=(0, 1))

    P[SL_S1["corr"]] += P[SL_S1_CORRB]
    P[SL_S1["cov"]] += P[SL_S1_COVB]
    S1_f3 = np.array([P[SL_S1["corr"]], P[SL_S1["cov"]], P[SL_S1["std"]],
                      P[SL_S1["z"]], P[SL_S1["dec"]], P[SL_S1["mean"]],
                      P[SL_S1["ret"]]])
    S2_f3 = np.array([P[SL_S2["corr"]], P[SL_S2["cov"]], P[SL_S2["std"]],
                      P[SL_S2["z"]], P[SL_S2["dec"]], P[SL_S2["mean"]] * 10.0,
                      P[SL_S2["ret"]]])
    S1_wmax = P[SL_S1_WMAX:SL_S1_WMAX + 7].copy()
    S2_wmax = P[SL_S2_WMAX:SL_S2_WMAX + 7].copy()
    S1_wmin = P[SL_S1_WMIN:SL_S1_WMIN + 7].copy()
    S2_wmin = P[SL_S2_WMIN:SL_S2_WMIN + 7].copy()
    S2_wm = P[SL_S2_WM:SL_S2_WM + 7].copy()
    S1_wm = S1_f3.copy()   # sum over windows of raw == window-sum totals

    # base group BN affines
    A_base = np.zeros(7); C_base = np.zeros(7)
    for g in range(7):
        alpha, bet = GAB[g]
        N = float(B * GCNT[g] * 3)
        S1 = S1_f3[g]
        S2 = S2_f3[g]
        mT = (alpha * S1 + bet * N) / N
        e2 = (alpha * alpha * S2 + 2 * alpha * bet * S1 + bet * bet * N) / N
        v = e2 - mT * mT
        a = gamma_f / np.sqrt(v + EPS)
        c = beta_f - a * mT
        A_base[g] = a * alpha
        C_base[g] = a * bet + c

    # second-level BN affines: si 0=wmean (from WM=3*wmean), 1=wmax, 2=wmin
    A2 = np.zeros((3, 7)); C2 = np.zeros((3, 7))
    for si in range(3):
        k = A_base * (1.0 / 3.0 if si == 0 else 1.0)
        off = C_base
        if si == 0:
            S1g = S1_wm; S2g = S2_wm
        elif si == 1:
            S1g = S1_wmax; S2g = S2_wmax
        else:
            S1g = S1_wmin; S2g = S2_wmin
        for grp_set, idxs in (("p1", range(6)), ("p2", [6])):
            Ntot = float(B * sum(GCNT[i] for i in idxs))
            m = sum(k[i] * S1g[i] + B * GCNT[i] * off[i] for i in idxs) / Ntot
            e2 = sum(k[i] ** 2 * S2g[i] + 2 * k[i] * off[i] * S1g[i]
                     + B * GCNT[i] * off[i] ** 2 for i in idxs) / Ntot
            v = e2 - m * m
            a2 = gamma_f / np.sqrt(v + EPS)
            c2 = beta_f - a2 * m
            for i in idxs:
                A2[si, i] = a2 * k[i]
                C2[si, i] = a2 * off[i] + c2

    # fold BN affine + wmean block into W1 over the 832 device columns
    W1A = np.zeros((32, 896), np.float64)
    for fm in range(165):
        g = GOF[fm]
        rf = FMAP[fm]
        for w in range(3):
            W1A[:30, 3 * fm + w] = (W1[:, 3 * rf + w] * A_base[g]
                                    + W1[:, 495 + rf] * A2[0, g])
        W1A[:30, 495 + fm] = W1[:, 495 + 165 + rf] * A2[1, g]
        W1A[:30, 660 + fm] = W1[:, 495 + 330 + rf] * A2[2, g]
    # constant side: sum over all reference columns of W1[:,rc] * C(rc)
    gof_ref = np.zeros(165, dtype=np.int64)
    gof_ref[FMAP] = GOF
    Cref = np.zeros(990)
    for rf in range(165):
        g = gof_ref[rf]
        for w in range(3):
            Cref[3 * rf + w] = C_base[g]
        for si in range(3):
            Cref[495 + 165 * si + rf] = C2[si, g]
    b1acc = b1.astype(np.float64) + W1.astype(np.float64) @ Cref

    # bias folded into W1A col 832 (device ones-row); u replicated per t-chunk
    W1A[:30, 832] = b1acc
    u = np.zeros(32, np.float64)
    u[:30] = wsc * W2[0]
    ub = np.tile(u.astype(np.float32), 4)[None, :].repeat(PT, axis=0)
    ub = np.ascontiguousarray(ub)
    c0 = np.float64(wsc * float(b2[0]) + bsc)

    in2 = [{"raw_in": raws[c],
            "w1t_in": np.ascontiguousarray(W1A.T.astype(np.float32)),
            "ub_in": ub,
            "id_in": np.eye(PT, dtype=np.float32)} for c in range(NCORES)]
    r2 = run_bass_kernel_spmd(nc2, in2, core_ids=list(range(NCORES)))
    if r2.exec_time_ns is not None:
        LAST_EXEC_NS.append(("phase2", r2.exec_time_ns))
        if r2.instructions_and_trace:
            LAST_TRACES.append(r2.instructions_and_trace[1])
    NB = BS // 512
    ys = []
    for r in r2.results:
        yb = r["y_out"].astype(np.float64).reshape(PT, NB, 4)   # [p, n, t]
        ys.append(np.transpose(yb, (1, 2, 0)).reshape(-1) + c0)
    y = np.concatenate(ys)
    return y.astype(np.float32)
